# revision 45
# baseline (speedup 1.0000x reference)
"""Trainium2 Bass kernel for BayesianGaussianMixture argmax responsibilities.

Math: wlp[n,k] = C[k] - ||X[n] @ P'_k - muP'_k||^2 ; out[n] = argmax_k wlp[n,k]
with P' = P/sqrt2, muP' = (mu_k @ P_k)/sqrt2. The digamma / stick-breaking /
log_det machinery folds into C[k] on the host (fp64).

fp32r matmuls run at 1 cycle/output-col (width >= 256) but round operands to
12 mantissa bits. fp32-grade accuracy comes from a host-side r12 hi/lo split
computed as three accumulated products (dropping Xlo@Plo ~ 2^-26):
  y = X@Phi + Xhi@Plo + mu-rows,   X = Xhi + Xlo (exact)
P is upper-triangular, so columns e<32 only need contraction rows d<32; the
three terms for those columns pack into ONE 98-row matmul. Columns e>=32 use
a 128-row X-exact matmul plus a 66-row correction matmul (Plo + mean rows).

Per-tile device pipeline (128 samples, psum = 8 banks of [8k x (32L|32H)]):
  PE   : 24 matmuls of 256 cols (fp32r), mean term folded in as const rows
  Act  : 2x Square activations [128,2048] psum->sbuf (ysq) — the bottleneck
         engine; it runs 100% busy in steady state (1892 ns per half-tile)
  Pool : pairwise folds of the H half: 32 -> 16 -> 8 cols per component
  DVE  : reduce(L-half) -> sqL; reduce([foldedH, -C[k], sqL], negate) -> wlp
         (the -C[k] column is pre-staged, so the reduce emits wlp directly);
         max8 + max_index -> argmax

Cost-model notes baked into the design:
  - matmul cost = output cols x 0.42ns (contraction depth is free, fp32r
    needs width >= 256 for the 1 cyc/col rate) -> pack all three precision
    terms into the contraction dim, bank-aligned 256-col outputs
  - only Act (0.83 ns/el) and DVE (1.04) can read PSUM, and an instruction
    may have at most ONE non-scalar PSUM input -> DVE cannot square
  - dependencies are tracked per-TILE -> stage independently-consumed
    operand pieces as separate tiles, spread over the 3 DMA queues
    (SP/Act HWDGE + Pool SWDGE), tile-0-critical pieces first
Sharding: data-parallel over N across 8 cores (4096 rows each); params
replicated. Verified 0/32768 argmax mismatches vs the fp32 jax reference.
"""

import os
import sys

import numpy as np

for _p in ("/opt/trn_rl_repo", "/opt/pypackages"):
    if _p not in sys.path and os.path.isdir(_p):
        sys.path.append(_p)

N, D, K = 32768, 64, 64
N_CORES = 8
N_SHARD = N // N_CORES          # 4096
TILES = N_SHARD // 128          # 32 tiles of 128 samples
NXCH = 4                        # X staged in 4 chunks of 1024 cols
EL = 32                         # e-split: L = e<32 (d<32 suffices), H = e>=32
YHC = 10                        # yh cols per k: 8 foldedH + 1 (-cb) + 1 (sqL)
LAST_TILE_SPLIT = True          # last tile: split drain (see loop)
T0_OCT_ACTS = False             # tile 0: per-octet Squares (hurts: lead-in is DMA-bound)
QPLAN = {'rlB': 'scalar', 'rhB': 'sync', 'rcB': 'gpsimd', 'rcA': 'gpsimd'}
XCH_PLAN = (256, 256) + (512,) * 7
TRACE = False                   # set by test harness

_CACHE = {}


def _digamma(x):
    """float64 digamma, valid for x > 0 (recurrence + asymptotic series)."""
    x = np.array(x, dtype=np.float64, copy=True)
    res = np.zeros_like(x)
    for _ in range(8):
        m = x < 6.0
        if not m.any():
            break
        res[m] -= 1.0 / x[m]
        x[m] += 1.0
    inv = 1.0 / x
    inv2 = inv * inv
    res += (
        np.log(x)
        - 0.5 * inv
        - inv2 * (1.0 / 12.0 - inv2 * (1.0 / 120.0 - inv2 * (1.0 / 252.0 - inv2 / 240.0)))
    )
    return res


def _r12(v):
    """HW fp32r rounding: round-to-nearest-even, 12 mantissa bits."""
    v = np.asarray(v, dtype=np.float32)
    m, e = np.frexp(v)
    return np.ldexp(np.round(m * 4096.0) / 4096.0, e).astype(np.float32)


def _host_prep(X, means, precisions_chol, weight_concentration, degrees_of_freedom,
               mean_precision):
    P = precisions_chol.astype(np.float64)            # [K, D, D] upper-tri
    mu = means.astype(np.float64)                     # [K, D]
    dof = degrees_of_freedom.astype(np.float64)       # [K]
    mp = mean_precision.astype(np.float64)            # [K]
    wc = weight_concentration.astype(np.float64)      # [2, K]

    log_det = np.log(np.diagonal(P, axis1=1, axis2=2)).sum(axis=1)      # [K]
    muP = np.einsum("kd,kde->ke", mu, P)                                # [K, D]
    i = np.arange(D, dtype=np.float64)
    log_lambda = D * np.log(2.0) + _digamma(
        0.5 * (dof[None, :] - i[:, None])).sum(axis=0)                  # [K]
    a, b = wc[0], wc[1]
    dsum = _digamma(a + b)
    log_w = _digamma(a) - dsum + np.concatenate(
        [[0.0], np.cumsum(_digamma(b) - dsum)[:-1]])
    const = (
        log_det
        - 0.5 * D * np.log(2.0 * np.pi)
        - 0.5 * D * np.log(dof)
        + 0.5 * (log_lambda - D / mp)
        + log_w
    )                                                                    # [K]

    s = 1.0 / np.sqrt(2.0)
    pcols = (P * s).transpose(1, 0, 2).reshape(D, K * D)  # [d, k*64+e] fp64
    muA = (-muP * s).reshape(K * D)                       # [k*64+e] fp64
    xt = X.T.astype(np.float32)                           # [64, N]
    return xt, pcols, muA, const


def _host_build(xt, pcols, muA, const):
    """Build the device operand tensors (all r12-gridded where f32r-loaded)."""
    phi = _r12(pcols.astype(np.float32))                       # [64, 4096]
    plo = _r12((pcols - phi).astype(np.float32))               # [64, 4096]
    muhi = _r12(muA.astype(np.float32))                        # [4096]
    mulo = _r12((muA - muhi).astype(np.float32))               # [4096]
    xhi = _r12(xt)                                             # [64, N]
    xlo = _r12(xt - xhi)                                       # [64, N]
    ones = np.ones((1, xt.shape[1]), np.float32)

    # column selections per octet: L = (ksub, e<32), H = (ksub, e>=32)
    lsel, hsel = [], []
    for o in range(8):
        for ks in range(8):
            base = (o * 8 + ks) * 64
            lsel.extend(range(base, base + EL))
            hsel.extend(range(base + EL, base + 64))
    lsel = np.array(lsel); hsel = np.array(hsel)               # [2048] each

    rl = np.concatenate([phi[:EL, lsel], phi[:EL, lsel], plo[:EL, lsel],
                         muhi[None, lsel], mulo[None, lsel]], axis=0)   # [98,2048]
    rh = np.concatenate([phi[:, hsel], phi[:, hsel]], axis=0)           # [128,2048]
    rc = np.concatenate([plo[:, hsel], muhi[None, hsel],
                         mulo[None, hsel]], axis=0)                     # [66,2048]

    xl = np.concatenate([xhi[:EL], xlo[:EL], xhi[:EL], ones, ones], axis=0)
    xh = np.concatenate([xhi, xlo], axis=0)                             # [128,N]
    xc = np.concatenate([xhi, ones, ones], axis=0)                      # [66,N]

    yhinit = np.zeros((128, K * YHC), np.float32)
    yhinit[:, 8::YHC] = -const.astype(np.float32)[None, :]              # -C[k]
    return rl, rh, rc, xl, xh, xc, yhinit


def _build_nc():
    import concourse.bass as bass
    import concourse.mybir as mybir
    import concourse.tile as tile
    from concourse import bacc
    from contextlib import ExitStack

    f32 = mybir.dt.float32
    f32r = mybir.dt.float32r
    u32 = mybir.dt.uint32
    ts = bass.ts
    Sq = mybir.ActivationFunctionType.Square
    ADD = mybir.AluOpType.add

    nc = bacc.Bacc("TRN2", target_bir_lowering=False, debug=False)
    rl_d = nc.dram_tensor("rl", [98, 2048], f32r, kind="ExternalInput")
    rh_d = nc.dram_tensor("rh", [128, 2048], f32r, kind="ExternalInput")
    rc_d = nc.dram_tensor("rc", [66, 2048], f32r, kind="ExternalInput")
    xl_d = nc.dram_tensor("xl", [98, N_SHARD], f32r, kind="ExternalInput")
    xh_d = nc.dram_tensor("xh", [128, N_SHARD], f32r, kind="ExternalInput")
    xc_d = nc.dram_tensor("xc", [66, N_SHARD], f32r, kind="ExternalInput")
    yhinit_d = nc.dram_tensor("yhinit", [128, K * YHC], f32, kind="ExternalInput")
    idxa_d = nc.dram_tensor("idxa", [128, TILES * 4], u32, kind="ExternalOutput")
    idxb_d = nc.dram_tensor("idxb", [128, TILES * 4], u32, kind="ExternalOutput")

    with tile.TileContext(nc) as tc, ExitStack() as ctx:
        consts = ctx.enter_context(tc.tile_pool(name="consts", bufs=1))
        psp = ctx.enter_context(tc.tile_pool(name="psp", bufs=1, space="PSUM"))
        ysqp = ctx.enter_context(tc.tile_pool(name="ysqp", bufs=2))
        scrp = ctx.enter_context(tc.tile_pool(name="scrp", bufs=2))
        smalls = ctx.enter_context(tc.tile_pool(name="smalls", bufs=4))

        def stage(qeng, dram_ap, shape, tg, dt=f32r):
            t = consts.tile(shape, dt, tag=tg)
            qeng.dma_start(t[:], dram_ap)
            return t

        # Spread staging over the three DGE queues (SP / Act HWDGE, Pool
        # SWDGE). Dependencies are tracked per-TILE, so every
        # independently-consumed piece gets its own tile: R operands are
        # split into per-half tiles (octets 0-3 / 4-7) so tile-0's first
        # matmuls only wait on the A-halves.
        # X chunk plan: small chunks first, then 512s.
        XCH = list(XCH_PLAN)
        xoff = np.cumsum([0] + XCH)
        t2ch = []                        # tile index -> (chunk, offset)
        for ci, w in enumerate(XCH):
            t2ch += [(ci, t * 128) for t in range(w // 128)]

        # octet-0 R slices first (one per queue), then the rest of half A
        # (octets 1-3), then half B (octets 4-7).
        xl_t = [stage(nc.sync, xl_d[:, 0:256], [98, 256], "xl0")]
        rl0 = stage(nc.sync, rl_d[:, 0:256], [98, 256], "rl0")
        rh0 = stage(nc.scalar, rh_d[:, 0:256], [128, 256], "rh0")
        xh_t = [stage(nc.gpsimd, xh_d[:, 0:256], [128, 256], "xh0")]
        rc0 = stage(nc.scalar, rc_d[:, 0:256], [66, 256], "rc0")
        xc_t = [stage(nc.gpsimd, xc_d[:, 0:256], [66, 256], "xc0")]
        rlA = stage(nc.sync, rl_d[:, 256:1024], [98, 768], "rlA")
        rhA = stage(nc.scalar, rh_d[:, 256:1024], [128, 768], "rhA")
        rcA = stage(getattr(nc, QPLAN["rcA"]), rc_d[:, 256:1024], [66, 768], "rcA")
        xl_t.append(stage(nc.sync, xl_d[:, 256:512], [98, 256], "xl1"))
        xh_t.append(stage(nc.gpsimd, xh_d[:, 256:512], [128, 256], "xh1"))
        xc_t.append(stage(nc.gpsimd, xc_d[:, 256:512], [66, 256], "xc1"))
        rlB = stage(getattr(nc, QPLAN["rlB"]), rl_d[:, ts(1, 1024)], [98, 1024], "rlB")
        rhB = stage(getattr(nc, QPLAN["rhB"]), rh_d[:, ts(1, 1024)], [128, 1024], "rhB")
        rcB = stage(getattr(nc, QPLAN["rcB"]), rc_d[:, ts(1, 1024)], [66, 1024], "rcB")
        yh_t = [stage(nc.gpsimd, yhinit_d[:], [128, K * YHC], f"yh{i}", f32)
                for i in range(2)]
        for ci in range(2, len(XCH)):
            o0, w = int(xoff[ci]), XCH[ci]
            xl_t.append(stage(nc.sync, xl_d[:, o0:o0 + w], [98, w], f"xl{ci}"))
            xh_t.append(stage(nc.sync, xh_d[:, o0:o0 + w], [128, w], f"xh{ci}"))
            xc_t.append(stage(nc.gpsimd, xc_d[:, o0:o0 + w], [66, w],
                              f"xc{ci}"))
        idx0 = consts.tile([128, TILES * 4], u32, tag="idx0")
        idx1 = consts.tile([128, TILES * 4], u32, tag="idx1")
        idxl = consts.tile([128, 8], u32, tag="idxl")
        idx_t = [idx0, idx1]

        def rslice(rlist, oc):
            """R operand slice for octet `half*4+oc` given (oct0, A, B)."""
            r0, rA, rB, half = rlist
            if half == 1:
                return rB[:, ts(oc, 256)]
            return r0[:] if oc == 0 else rA[:, ts(oc - 1, 256)]

        for t_i in range(TILES):
            ci, co = t2ch[t_i]
            xl = xl_t[ci][:, co:co + 128]
            xh = xh_t[ci][:, co:co + 128]
            xc = xc_t[ci][:, co:co + 128]

            ysq = ysqp.tile([128, 4096], f32, tag="ysq")
            for half in range(2):
                ps = psp.tile([128, 2048], f32, tag=f"ps{half}")
                for oc in range(4):
                    lout = ps[:, oc * 512:oc * 512 + 256]
                    hout = ps[:, oc * 512 + 256:oc * 512 + 512]
                    nc.tensor.matmul(lout, xl, rslice((rl0, rlA, rlB, half), oc),
                                     start=True, stop=True)
                    nc.tensor.matmul(hout, xh, rslice((rh0, rhA, rhB, half), oc),
                                     start=True, stop=False)
                    nc.tensor.matmul(hout, xc,
                                     rslice((rc0, rcA, rcB, half), oc),
                                     start=False, stop=True)
                if t_i == 0 and T0_OCT_ACTS:
                    # Tile 0: per-octet Squares so Act starts as soon as the
                    # first octet's staged operands land (the R tensors are
                    # still streaming in during the lead-in).
                    for oc in range(4):
                        nc.scalar.activation(
                            ysq[:, half * 2048 + oc * 512:
                                half * 2048 + (oc + 1) * 512],
                            ps[:, ts(oc, 512)], Sq)
                else:
                    nc.scalar.activation(ysq[:, ts(half, 2048)], ps[:], Sq)

            def yq():   # [128, oct(8), ksub(16: 0-7=L, 8-15=H), e(32)]
                return ysq[:].rearrange("p (o q e) -> p o q e", o=8, q=16, e=EL)

            def sq4():  # fold scratch as [128, oct(8), ksub(8), e(16)]
                return scr[:].rearrange("p (o q e) -> p o q e", o=8, q=8, e=16)

            yh = yh_t[t_i % 2]
            yhv = yh[:].rearrange("p (k c) -> p k c", c=YHC)
            wlp = smalls.tile([128, K], f32, tag="wlp")
            scr = scrp.tile([128, 1024], f32, tag="scr")
            if t_i == TILES - 1 and LAST_TILE_SPLIT:
                # Last tile: everything split per psum-half, fold2 on DVE —
                # shortens the serial drain chain after the final Square.
                nc.gpsimd.tensor_tensor(sq4()[:, 0:4, :, :],
                                        yq()[:, 0:4, 8:16, 0:16],
                                        yq()[:, 0:4, 8:16, 16:32], op=ADD)
                nc.vector.tensor_reduce(yhv[:, 0:32, 9:10],
                                        yq()[:, 0:4, 0:8, :],
                                        axis=mybir.AxisListType.X, op=ADD)
                nc.gpsimd.tensor_tensor(sq4()[:, 4:8, :, :],
                                        yq()[:, 4:8, 8:16, 0:16],
                                        yq()[:, 4:8, 8:16, 16:32], op=ADD)
                nc.vector.tensor_reduce(yhv[:, 32:64, 9:10],
                                        yq()[:, 4:8, 0:8, :],
                                        axis=mybir.AxisListType.X, op=ADD)
                nc.vector.tensor_tensor(
                    yh[:].rearrange("p (o q c) -> p o q c", o=8, q=8)[:, :, :, 0:8],
                    sq4()[:, :, :, 0:8], sq4()[:, :, :, 8:16], op=ADD)
            else:
                nc.gpsimd.tensor_tensor(sq4(), yq()[:, :, 8:16, 0:16],
                                        yq()[:, :, 8:16, 16:32], op=ADD)
                nc.gpsimd.tensor_tensor(
                    yh[:].rearrange("p (o q c) -> p o q c", o=8, q=8)[:, :, :, 0:8],
                    sq4()[:, :, :, 0:8], sq4()[:, :, :, 8:16], op=ADD)
                nc.vector.tensor_reduce(yhv[:, :, 9:10], yq()[:, :, 0:8, :],
                                        axis=mybir.AxisListType.X, op=ADD)
            nc.vector.tensor_reduce(wlp[:], yhv, axis=mybir.AxisListType.X,
                                    op=ADD, negate=True)
            mx8 = smalls.tile([128, 8], f32, tag="mx8")
            nc.vector.max(mx8[:], wlp[:])
            nc.vector.max_index(
                idxl[:] if t_i == TILES - 1 else
                idx_t[t_i // 16][:, ts(t_i % 16, 8)], mx8[:], wlp[:])
            if t_i == TILES // 2 - 1:
                # first-half output overlaps the back half of the kernel
                nc.sync.dma_start(idxa_d[:], idx_t[0][:])
            elif t_i == TILES - 2:
                # all but the last tile's slice overlaps the drain
                nc.sync.dma_start(idxb_d[:, 0:120], idx_t[1][:, 0:120])

        nc.sync.dma_start(idxb_d[:, 120:128], idxl[:])
    nc.compile()
    return nc


def _make_in_maps(rl, rh, rc, xl, xh, xc, yhinit):
    in_maps = []
    for c in range(N_CORES):
        sl = slice(c * N_SHARD, (c + 1) * N_SHARD)
        in_maps.append({
            "rl": rl, "rh": rh, "rc": rc,
            "xl": np.ascontiguousarray(xl[:, sl]),
            "xh": np.ascontiguousarray(xh[:, sl]),
            "xc": np.ascontiguousarray(xc[:, sl]),
            "yhinit": yhinit,
        })
    return in_maps


def _run(nc, in_maps):
    from concourse.bass_utils import run_bass_kernel_spmd

    try:
        res = run_bass_kernel_spmd(nc, in_maps, core_ids=list(range(N_CORES)),
                                   trace=TRACE)
    except (ModuleNotFoundError, ImportError):
        res = run_bass_kernel_spmd(nc, in_maps, core_ids=list(range(N_CORES)))
    _CACHE["last_results"] = res
    return res


def _gather(res):
    out = np.empty(N, dtype=np.int32)
    for c in range(N_CORES):
        idx = np.concatenate([res.results[c]["idxa"], res.results[c]["idxb"]],
                             axis=1)                # [128, TILES*8] uint32
        sel = idx.reshape(128, TILES, 8)[:, :, 0]   # [128, TILES]
        out[c * N_SHARD:(c + 1) * N_SHARD] = sel.T.reshape(-1).astype(np.int32)
    return out


def _spot_check(out, xt, pcols, muA, const, n_probe=96, seed=0):
    """Host fp64 recompute of a few random rows; detects gross corruption
    (stale/garbage device output), not fp noise."""
    rng = np.random.default_rng(seed)
    rows = rng.choice(N, size=n_probe, replace=False)
    y = xt[:, rows].astype(np.float64).T @ pcols + muA[None, :]   # [n, K*D]
    sq = (y ** 2).reshape(-1, K, D).sum(axis=2)
    ref = (const[None, :] - sq).argmax(axis=1)
    return (out[rows] == ref).mean() >= 0.95


def kernel(X, means, precisions_chol, weight_concentration, degrees_of_freedom,
           mean_precision):
    xt, pcols, muA, const = _host_prep(
        X, means, precisions_chol, weight_concentration, degrees_of_freedom,
        mean_precision)
    rl, rh, rc, xl, xh, xc, yhinit = _host_build(xt, pcols, muA, const)
    if "nc" not in _CACHE:
        _CACHE["nc"] = _build_nc()
    in_maps = _make_in_maps(rl, rh, rc, xl, xh, xc, yhinit)

    out = None
    last_exc = None
    for attempt in range(3):
        try:
            res = _run(_CACHE["nc"], in_maps)
            out = _gather(res)
        except Exception as e:           # transient device fault: retry
            last_exc = e
            import time
            time.sleep(2.0)
            continue
        if (out >= 0).all() and (out < K).all() and _spot_check(
                out, xt, pcols, muA, const):
            break
    if out is None:
        raise last_exc
    return out


# revision 46
# speedup vs baseline: 1.0005x; 1.0005x over previous
"""Trainium2 Bass kernel for BayesianGaussianMixture argmax responsibilities.

Math: wlp[n,k] = C[k] - ||X[n] @ P'_k - muP'_k||^2 ; out[n] = argmax_k wlp[n,k]
with P' = P/sqrt2, muP' = (mu_k @ P_k)/sqrt2. The digamma / stick-breaking /
log_det machinery folds into C[k] on the host (fp64).

fp32r matmuls run at 1 cycle/output-col (width >= 256) but round operands to
12 mantissa bits. fp32-grade accuracy comes from a host-side r12 hi/lo split
computed as three accumulated products (dropping Xlo@Plo ~ 2^-26):
  y = X@Phi + Xhi@Plo + mu-rows,   X = Xhi + Xlo (exact)
P is upper-triangular, so columns e<32 only need contraction rows d<32; the
three terms for those columns pack into ONE 98-row matmul. Columns e>=32 use
a 128-row X-exact matmul plus a 66-row correction matmul (Plo + mean rows).

Per-tile device pipeline (128 samples, psum = 8 banks of [8k x (32L|32H)]):
  PE   : 24 matmuls of 256 cols (fp32r), mean term folded in as const rows
  Act  : 2x Square activations [128,2048] psum->sbuf (ysq) — the bottleneck
         engine; it runs 100% busy in steady state (1892 ns per half-tile)
  Pool : pairwise folds of the H half: 32 -> 16 -> 8 cols per component
  DVE  : reduce(L-half) -> sqL; reduce([foldedH, -C[k], sqL], negate) -> wlp
         (the -C[k] column is pre-staged, so the reduce emits wlp directly);
         max8 + max_index -> argmax

Cost-model notes baked into the design:
  - matmul cost = output cols x 0.42ns (contraction depth is free, fp32r
    needs width >= 256 for the 1 cyc/col rate) -> pack all three precision
    terms into the contraction dim, bank-aligned 256-col outputs
  - only Act (0.83 ns/el) and DVE (1.04) can read PSUM, and an instruction
    may have at most ONE non-scalar PSUM input -> DVE cannot square
  - dependencies are tracked per-TILE -> stage independently-consumed
    operand pieces as separate tiles, spread over the 3 DMA queues
    (SP/Act HWDGE + Pool SWDGE), tile-0-critical pieces first
Sharding: data-parallel over N across 8 cores (4096 rows each); params
replicated. Verified 0/32768 argmax mismatches vs the fp32 jax reference.
"""

import os
import sys

import numpy as np

for _p in ("/opt/trn_rl_repo", "/opt/pypackages"):
    if _p not in sys.path and os.path.isdir(_p):
        sys.path.append(_p)

N, D, K = 32768, 64, 64
N_CORES = 8
N_SHARD = N // N_CORES          # 4096
TILES = N_SHARD // 128          # 32 tiles of 128 samples
NXCH = 4                        # X staged in 4 chunks of 1024 cols
EL = 32                         # e-split: L = e<32 (d<32 suffices), H = e>=32
YHC = 10                        # yh cols per k: 8 foldedH + 1 (-cb) + 1 (sqL)
LAST_TILE_SPLIT = True          # last tile: split drain (see loop)
T0_OCT_ACTS = False             # tile 0: per-octet Squares (hurts: lead-in is DMA-bound)
QPLAN = {'rlB': 'scalar', 'rhB': 'sync', 'rcB': 'gpsimd', 'rcA': 'gpsimd'}
XCH_PLAN = (256, 256) + (512,) * 7
TRACE = False                   # set by test harness

_CACHE = {}


def _digamma(x):
    """float64 digamma, valid for x > 0 (recurrence + asymptotic series)."""
    x = np.array(x, dtype=np.float64, copy=True)
    res = np.zeros_like(x)
    for _ in range(8):
        m = x < 6.0
        if not m.any():
            break
        res[m] -= 1.0 / x[m]
        x[m] += 1.0
    inv = 1.0 / x
    inv2 = inv * inv
    res += (
        np.log(x)
        - 0.5 * inv
        - inv2 * (1.0 / 12.0 - inv2 * (1.0 / 120.0 - inv2 * (1.0 / 252.0 - inv2 / 240.0)))
    )
    return res


def _r12(v):
    """HW fp32r rounding: round-to-nearest-even, 12 mantissa bits."""
    v = np.asarray(v, dtype=np.float32)
    m, e = np.frexp(v)
    return np.ldexp(np.round(m * 4096.0) / 4096.0, e).astype(np.float32)


def _host_prep(X, means, precisions_chol, weight_concentration, degrees_of_freedom,
               mean_precision):
    P = precisions_chol.astype(np.float64)            # [K, D, D] upper-tri
    mu = means.astype(np.float64)                     # [K, D]
    dof = degrees_of_freedom.astype(np.float64)       # [K]
    mp = mean_precision.astype(np.float64)            # [K]
    wc = weight_concentration.astype(np.float64)      # [2, K]

    log_det = np.log(np.diagonal(P, axis1=1, axis2=2)).sum(axis=1)      # [K]
    muP = np.einsum("kd,kde->ke", mu, P)                                # [K, D]
    i = np.arange(D, dtype=np.float64)
    log_lambda = D * np.log(2.0) + _digamma(
        0.5 * (dof[None, :] - i[:, None])).sum(axis=0)                  # [K]
    a, b = wc[0], wc[1]
    dsum = _digamma(a + b)
    log_w = _digamma(a) - dsum + np.concatenate(
        [[0.0], np.cumsum(_digamma(b) - dsum)[:-1]])
    const = (
        log_det
        - 0.5 * D * np.log(2.0 * np.pi)
        - 0.5 * D * np.log(dof)
        + 0.5 * (log_lambda - D / mp)
        + log_w
    )                                                                    # [K]

    s = 1.0 / np.sqrt(2.0)
    pcols = (P * s).transpose(1, 0, 2).reshape(D, K * D)  # [d, k*64+e] fp64
    muA = (-muP * s).reshape(K * D)                       # [k*64+e] fp64
    xt = X.T.astype(np.float32)                           # [64, N]
    return xt, pcols, muA, const


def _host_build(xt, pcols, muA, const):
    """Build the device operand tensors (all r12-gridded where f32r-loaded)."""
    phi = _r12(pcols.astype(np.float32))                       # [64, 4096]
    plo = _r12((pcols - phi).astype(np.float32))               # [64, 4096]
    muhi = _r12(muA.astype(np.float32))                        # [4096]
    mulo = _r12((muA - muhi).astype(np.float32))               # [4096]
    xhi = _r12(xt)                                             # [64, N]
    xlo = _r12(xt - xhi)                                       # [64, N]
    ones = np.ones((1, xt.shape[1]), np.float32)

    # column selections per octet: L = (ksub, e<32), H = (ksub, e>=32)
    lsel, hsel = [], []
    for o in range(8):
        for ks in range(8):
            base = (o * 8 + ks) * 64
            lsel.extend(range(base, base + EL))
            hsel.extend(range(base + EL, base + 64))
    lsel = np.array(lsel); hsel = np.array(hsel)               # [2048] each

    rl = np.concatenate([phi[:EL, lsel], phi[:EL, lsel], plo[:EL, lsel],
                         muhi[None, lsel], mulo[None, lsel]], axis=0)   # [98,2048]
    rh = np.concatenate([phi[:, hsel], phi[:, hsel]], axis=0)           # [128,2048]
    rc = np.concatenate([plo[:, hsel], muhi[None, hsel],
                         mulo[None, hsel]], axis=0)                     # [66,2048]

    xl = np.concatenate([xhi[:EL], xlo[:EL], xhi[:EL], ones, ones], axis=0)
    xh = np.concatenate([xhi, xlo], axis=0)                             # [128,N]
    xc = np.concatenate([xhi, ones, ones], axis=0)                      # [66,N]

    yhinit = np.zeros((128, K * YHC), np.float32)
    yhinit[:, 8::YHC] = -const.astype(np.float32)[None, :]              # -C[k]
    return rl, rh, rc, xl, xh, xc, yhinit


def _build_nc():
    import concourse.bass as bass
    import concourse.mybir as mybir
    import concourse.tile as tile
    from concourse import bacc
    from contextlib import ExitStack

    f32 = mybir.dt.float32
    f32r = mybir.dt.float32r
    u32 = mybir.dt.uint32
    ts = bass.ts
    Sq = mybir.ActivationFunctionType.Square
    ADD = mybir.AluOpType.add

    nc = bacc.Bacc("TRN2", target_bir_lowering=False, debug=False)
    rl_d = nc.dram_tensor("rl", [98, 2048], f32r, kind="ExternalInput")
    rh_d = nc.dram_tensor("rh", [128, 2048], f32r, kind="ExternalInput")
    rc_d = nc.dram_tensor("rc", [66, 2048], f32r, kind="ExternalInput")
    xl_d = nc.dram_tensor("xl", [98, N_SHARD], f32r, kind="ExternalInput")
    xh_d = nc.dram_tensor("xh", [128, N_SHARD], f32r, kind="ExternalInput")
    xc_d = nc.dram_tensor("xc", [66, N_SHARD], f32r, kind="ExternalInput")
    yhinit_d = nc.dram_tensor("yhinit", [128, K * YHC], f32, kind="ExternalInput")
    idxa_d = nc.dram_tensor("idxa", [128, TILES * 4], u32, kind="ExternalOutput")
    idxb_d = nc.dram_tensor("idxb", [128, TILES * 4], u32, kind="ExternalOutput")

    with tile.TileContext(nc) as tc, ExitStack() as ctx:
        consts = ctx.enter_context(tc.tile_pool(name="consts", bufs=1))
        psp = ctx.enter_context(tc.tile_pool(name="psp", bufs=1, space="PSUM"))
        ysqp = ctx.enter_context(tc.tile_pool(name="ysqp", bufs=2))
        scrp = ctx.enter_context(tc.tile_pool(name="scrp", bufs=2))
        smalls = ctx.enter_context(tc.tile_pool(name="smalls", bufs=4))

        def stage(qeng, dram_ap, shape, tg, dt=f32r):
            t = consts.tile(shape, dt, tag=tg)
            qeng.dma_start(t[:], dram_ap)
            return t

        # Spread staging over the three DGE queues (SP / Act HWDGE, Pool
        # SWDGE). Dependencies are tracked per-TILE, so every
        # independently-consumed piece gets its own tile: R operands are
        # split into per-half tiles (octets 0-3 / 4-7) so tile-0's first
        # matmuls only wait on the A-halves.
        # X chunk plan: small chunks first, then 512s.
        XCH = list(XCH_PLAN)
        xoff = np.cumsum([0] + XCH)
        t2ch = []                        # tile index -> (chunk, offset)
        for ci, w in enumerate(XCH):
            t2ch += [(ci, t * 128) for t in range(w // 128)]

        # octet-0 R slices first (one per queue), then the rest of half A
        # (octets 1-3), then half B (octets 4-7).
        xl_t = [stage(nc.sync, xl_d[:, 0:256], [98, 256], "xl0")]
        rl0 = stage(nc.sync, rl_d[:, 0:256], [98, 256], "rl0")
        rh0 = stage(nc.scalar, rh_d[:, 0:256], [128, 256], "rh0")
        xh_t = [stage(nc.gpsimd, xh_d[:, 0:256], [128, 256], "xh0")]
        rc0 = stage(nc.scalar, rc_d[:, 0:256], [66, 256], "rc0")
        xc_t = [stage(nc.gpsimd, xc_d[:, 0:256], [66, 256], "xc0")]
        rlA = stage(nc.sync, rl_d[:, 256:1024], [98, 768], "rlA")
        rhA = stage(nc.scalar, rh_d[:, 256:1024], [128, 768], "rhA")
        rcA = stage(getattr(nc, QPLAN["rcA"]), rc_d[:, 256:1024], [66, 768], "rcA")
        xl_t.append(stage(nc.sync, xl_d[:, 256:512], [98, 256], "xl1"))
        xh_t.append(stage(nc.gpsimd, xh_d[:, 256:512], [128, 256], "xh1"))
        xc_t.append(stage(nc.gpsimd, xc_d[:, 256:512], [66, 256], "xc1"))
        rlB = stage(getattr(nc, QPLAN["rlB"]), rl_d[:, ts(1, 1024)], [98, 1024], "rlB")
        rhB = stage(getattr(nc, QPLAN["rhB"]), rh_d[:, ts(1, 1024)], [128, 1024], "rhB")
        rcB = stage(getattr(nc, QPLAN["rcB"]), rc_d[:, ts(1, 1024)], [66, 1024], "rcB")
        yh_t = [stage(nc.gpsimd, yhinit_d[:], [128, K * YHC], f"yh{i}", f32)
                for i in range(2)]
        for ci in range(2, len(XCH)):
            o0, w = int(xoff[ci]), XCH[ci]
            xl_t.append(stage(nc.sync, xl_d[:, o0:o0 + w], [98, w], f"xl{ci}"))
            xh_t.append(stage(nc.sync, xh_d[:, o0:o0 + w], [128, w], f"xh{ci}"))
            xc_t.append(stage(nc.gpsimd, xc_d[:, o0:o0 + w], [66, w],
                              f"xc{ci}"))
        idx0 = consts.tile([128, TILES * 4], u32, tag="idx0")
        idx1 = consts.tile([128, TILES * 4], u32, tag="idx1")
        idxl = consts.tile([128, 8], u32, tag="idxl")
        idx_t = [idx0, idx1]

        def rslice(rlist, oc):
            """R operand slice for octet `half*4+oc` given (oct0, A, B)."""
            r0, rA, rB, half = rlist
            if half == 1:
                return rB[:, ts(oc, 256)]
            return r0[:] if oc == 0 else rA[:, ts(oc - 1, 256)]

        for t_i in range(TILES):
            ci, co = t2ch[t_i]
            xl = xl_t[ci][:, co:co + 128]
            xh = xh_t[ci][:, co:co + 128]
            xc = xc_t[ci][:, co:co + 128]

            ysq = ysqp.tile([128, 4096], f32, tag="ysq")
            for half in range(2):
                ps = psp.tile([128, 2048], f32, tag=f"ps{half}")
                if t_i == 0:
                    # Tile 0: emit all L matmuls first — the C operands
                    # arrive last during staging and would head-of-line
                    # block the in-order PE queue.
                    for oc in range(4):
                        nc.tensor.matmul(ps[:, oc * 512:oc * 512 + 256], xl,
                                         rslice((rl0, rlA, rlB, half), oc),
                                         start=True, stop=True)
                else:
                    for oc in range(4):
                        nc.tensor.matmul(ps[:, oc * 512:oc * 512 + 256], xl,
                                         rslice((rl0, rlA, rlB, half), oc),
                                         start=True, stop=True)
                        hout = ps[:, oc * 512 + 256:oc * 512 + 512]
                        nc.tensor.matmul(hout, xh,
                                         rslice((rh0, rhA, rhB, half), oc),
                                         start=True, stop=False)
                        nc.tensor.matmul(hout, xc,
                                         rslice((rc0, rcA, rcB, half), oc),
                                         start=False, stop=True)
                if t_i == 0:
                    for oc in range(4):
                        hout = ps[:, oc * 512 + 256:oc * 512 + 512]
                        nc.tensor.matmul(hout, xh,
                                         rslice((rh0, rhA, rhB, half), oc),
                                         start=True, stop=False)
                        nc.tensor.matmul(hout, xc,
                                         rslice((rc0, rcA, rcB, half), oc),
                                         start=False, stop=True)
                if t_i == 0 and T0_OCT_ACTS:
                    # Tile 0: per-octet Squares so Act starts as soon as the
                    # first octet's staged operands land (the R tensors are
                    # still streaming in during the lead-in).
                    for oc in range(4):
                        nc.scalar.activation(
                            ysq[:, half * 2048 + oc * 512:
                                half * 2048 + (oc + 1) * 512],
                            ps[:, ts(oc, 512)], Sq)
                else:
                    nc.scalar.activation(ysq[:, ts(half, 2048)], ps[:], Sq)

            def yq():   # [128, oct(8), ksub(16: 0-7=L, 8-15=H), e(32)]
                return ysq[:].rearrange("p (o q e) -> p o q e", o=8, q=16, e=EL)

            def sq4():  # fold scratch as [128, oct(8), ksub(8), e(16)]
                return scr[:].rearrange("p (o q e) -> p o q e", o=8, q=8, e=16)

            yh = yh_t[t_i % 2]
            yhv = yh[:].rearrange("p (k c) -> p k c", c=YHC)
            wlp = smalls.tile([128, K], f32, tag="wlp")
            scr = scrp.tile([128, 1024], f32, tag="scr")
            if t_i == TILES - 1 and LAST_TILE_SPLIT:
                # Last tile: everything split per psum-half, fold2 on DVE —
                # shortens the serial drain chain after the final Square.
                nc.gpsimd.tensor_tensor(sq4()[:, 0:4, :, :],
                                        yq()[:, 0:4, 8:16, 0:16],
                                        yq()[:, 0:4, 8:16, 16:32], op=ADD)
                nc.vector.tensor_reduce(yhv[:, 0:32, 9:10],
                                        yq()[:, 0:4, 0:8, :],
                                        axis=mybir.AxisListType.X, op=ADD)
                nc.gpsimd.tensor_tensor(sq4()[:, 4:8, :, :],
                                        yq()[:, 4:8, 8:16, 0:16],
                                        yq()[:, 4:8, 8:16, 16:32], op=ADD)
                nc.vector.tensor_reduce(yhv[:, 32:64, 9:10],
                                        yq()[:, 4:8, 0:8, :],
                                        axis=mybir.AxisListType.X, op=ADD)
                nc.vector.tensor_tensor(
                    yh[:].rearrange("p (o q c) -> p o q c", o=8, q=8)[:, :, :, 0:8],
                    sq4()[:, :, :, 0:8], sq4()[:, :, :, 8:16], op=ADD)
            else:
                nc.gpsimd.tensor_tensor(sq4(), yq()[:, :, 8:16, 0:16],
                                        yq()[:, :, 8:16, 16:32], op=ADD)
                nc.gpsimd.tensor_tensor(
                    yh[:].rearrange("p (o q c) -> p o q c", o=8, q=8)[:, :, :, 0:8],
                    sq4()[:, :, :, 0:8], sq4()[:, :, :, 8:16], op=ADD)
                nc.vector.tensor_reduce(yhv[:, :, 9:10], yq()[:, :, 0:8, :],
                                        axis=mybir.AxisListType.X, op=ADD)
            nc.vector.tensor_reduce(wlp[:], yhv, axis=mybir.AxisListType.X,
                                    op=ADD, negate=True)
            mx8 = smalls.tile([128, 8], f32, tag="mx8")
            nc.vector.max(mx8[:], wlp[:])
            nc.vector.max_index(
                idxl[:] if t_i == TILES - 1 else
                idx_t[t_i // 16][:, ts(t_i % 16, 8)], mx8[:], wlp[:])
            if t_i == TILES // 2 - 1:
                # first-half output overlaps the back half of the kernel
                nc.sync.dma_start(idxa_d[:], idx_t[0][:])
            elif t_i == TILES - 2:
                # all but the last tile's slice overlaps the drain
                nc.sync.dma_start(idxb_d[:, 0:120], idx_t[1][:, 0:120])

        nc.sync.dma_start(idxb_d[:, 120:128], idxl[:])
    nc.compile()
    return nc


def _make_in_maps(rl, rh, rc, xl, xh, xc, yhinit):
    in_maps = []
    for c in range(N_CORES):
        sl = slice(c * N_SHARD, (c + 1) * N_SHARD)
        in_maps.append({
            "rl": rl, "rh": rh, "rc": rc,
            "xl": np.ascontiguousarray(xl[:, sl]),
            "xh": np.ascontiguousarray(xh[:, sl]),
            "xc": np.ascontiguousarray(xc[:, sl]),
            "yhinit": yhinit,
        })
    return in_maps


def _run(nc, in_maps):
    from concourse.bass_utils import run_bass_kernel_spmd

    try:
        res = run_bass_kernel_spmd(nc, in_maps, core_ids=list(range(N_CORES)),
                                   trace=TRACE)
    except (ModuleNotFoundError, ImportError):
        res = run_bass_kernel_spmd(nc, in_maps, core_ids=list(range(N_CORES)))
    _CACHE["last_results"] = res
    return res


def _gather(res):
    out = np.empty(N, dtype=np.int32)
    for c in range(N_CORES):
        idx = np.concatenate([res.results[c]["idxa"], res.results[c]["idxb"]],
                             axis=1)                # [128, TILES*8] uint32
        sel = idx.reshape(128, TILES, 8)[:, :, 0]   # [128, TILES]
        out[c * N_SHARD:(c + 1) * N_SHARD] = sel.T.reshape(-1).astype(np.int32)
    return out


def _spot_check(out, xt, pcols, muA, const, n_probe=96, seed=0):
    """Host fp64 recompute of a few random rows; detects gross corruption
    (stale/garbage device output), not fp noise."""
    rng = np.random.default_rng(seed)
    rows = rng.choice(N, size=n_probe, replace=False)
    y = xt[:, rows].astype(np.float64).T @ pcols + muA[None, :]   # [n, K*D]
    sq = (y ** 2).reshape(-1, K, D).sum(axis=2)
    ref = (const[None, :] - sq).argmax(axis=1)
    return (out[rows] == ref).mean() >= 0.95


def kernel(X, means, precisions_chol, weight_concentration, degrees_of_freedom,
           mean_precision):
    xt, pcols, muA, const = _host_prep(
        X, means, precisions_chol, weight_concentration, degrees_of_freedom,
        mean_precision)
    rl, rh, rc, xl, xh, xc, yhinit = _host_build(xt, pcols, muA, const)
    if "nc" not in _CACHE:
        _CACHE["nc"] = _build_nc()
    in_maps = _make_in_maps(rl, rh, rc, xl, xh, xc, yhinit)

    out = None
    last_exc = None
    for attempt in range(3):
        try:
            res = _run(_CACHE["nc"], in_maps)
            out = _gather(res)
        except Exception as e:           # transient device fault: retry
            last_exc = e
            import time
            time.sleep(2.0)
            continue
        if (out >= 0).all() and (out < K).all() and _spot_check(
                out, xt, pcols, muA, const):
            break
    if out is None:
        raise last_exc
    return out
